# revision 15
# baseline (speedup 1.0000x reference)
"""Trainium2 Bass kernel: batched Cartesian -> Jacobi coordinate transform.

Math (per batch element, N = 11 bodies, D = 3):
    M[i]   = cumsum(m)[i]
    COM[i] = (sum_{j<=i} m[j] x[j]) / M[i]
           = a[i] * COM[i-1] + b[i] * x[i],  b[i] = m[i]/M[i], a[i] = 1 - b[i]
    xj[0]  = COM[N-1]          (x is q or v)
    xj[i]  = x[i] - COM[i-1]   (i >= 1)
    mu     = G * M

a[0] == 0 exactly (b[0] = m[0]/M[0] = 1), so the recurrence self-resets at
each batch element's first body.  That lets the whole tile's COM computation
run as a single DVE tensor_tensor_scan per coordinate component d (op0=mult,
op1=add, strided 2-D access pattern), with the reset riding along in data0.

Layout: batch on the 128 SBUF partitions, G batch elements per partition per
tile; each batch element's [N, D] block stays contiguous along the free dim,
so all HBM DMAs are fully contiguous.

Sharding: pure data-parallel over the batch dim across 8 NeuronCores.
"""

import numpy as np

import concourse.bacc as bacc
import concourse.bass as bass
import concourse.mybir as mybir
import concourse.tile as tile
from concourse.bass_utils import run_bass_kernel_spmd

G_GRAV = 2.959122082855911e-4
B_FULL, N, D = 524288, 11, 3
N_CORES = 8
P = 128  # SBUF partitions
BC = B_FULL // N_CORES  # batch per core
G_DEFAULT = 64  # batch elements per partition per tile


class _Bacc(bacc.Bacc):
    """Bacc whose activation-table selection is pinned to
    `natural_log_exp_and_others` — the one set that really contains Ln, Exp,
    Identity and Copy — so the per-tile Ln/Exp + Copy mix doesn't thrash ACT
    table loads (the stock greedy pass alternates natural_log <->
    exp_and_others, ~2.7us per reload)."""

    def insert_act_table_loads(self):
        import bass_rust as _bass_rust
        from concourse.hw_specs import get_activation_tables

        has_activation = any(
            isinstance(i, mybir.InstActivation)
            for b in self.main_func.blocks
            for i in b.instructions
        )
        if not has_activation:
            return
        AF = mybir.ActivationFunctionType
        keep = "natural_log_exp_and_others"
        strip = {AF.Ln, AF.Exp, AF.Identity, AF.Copy}
        tables = [
            (name, funcs if name == keep else (funcs - strip))
            for name, funcs in get_activation_tables(self.m.arch).items()
        ]
        _bass_rust.insert_act_table_loads(self, tables)


def build_nc(bc: int, g: int, reps: int = 1) -> bass.Bass:
    """Build the per-core Bass program for a batch shard of `bc` elements.

    reps > 1 repeats the whole computation (same I/O) for differential
    wall-clock timing; results are unchanged."""
    nt = bc // (P * g)
    assert nt * P * g == bc, (bc, g)
    f32 = mybir.dt.float32
    AF = mybir.ActivationFunctionType
    OP = mybir.AluOpType
    FN = g * N
    FND = g * N * D

    nc = _Bacc("TRN2")
    m_in = nc.dram_tensor("m", [bc, N], f32, kind="ExternalInput")
    q_in = nc.dram_tensor("q", [bc, N, D], f32, kind="ExternalInput")
    v_in = nc.dram_tensor("v", [bc, N, D], f32, kind="ExternalInput")
    qj_out = nc.dram_tensor("qj", [bc, N, D], f32, kind="ExternalOutput")
    vj_out = nc.dram_tensor("vj", [bc, N, D], f32, kind="ExternalOutput")
    mu_out = nc.dram_tensor("mu", [bc, N], f32, kind="ExternalOutput")

    # [t, p, free] views; batch index b = (t*128 + p)*g + gi  (fully contiguous)
    mv = m_in[:].rearrange("(t p gi) n -> t p (gi n)", t=nt, p=P, gi=g)
    qv = q_in[:].rearrange("(t p gi) n d -> t p (gi n d)", t=nt, p=P, gi=g)
    vv = v_in[:].rearrange("(t p gi) n d -> t p (gi n d)", t=nt, p=P, gi=g)
    qjv = qj_out[:].rearrange("(t p gi) n d -> t p (gi n d)", t=nt, p=P, gi=g)
    vjv = vj_out[:].rearrange("(t p gi) n d -> t p (gi n d)", t=nt, p=P, gi=g)
    muv = mu_out[:].rearrange("(t p gi) n -> t p (gi n)", t=nt, p=P, gi=g)

    with tile.TileContext(nc) as tc:
        from contextlib import ExitStack

        with ExitStack() as ctx:
            io_q = ctx.enter_context(tc.tile_pool(name="io_q", bufs=3))
            io_v = ctx.enter_context(tc.tile_pool(name="io_v", bufs=3))
            io_m = ctx.enter_context(tc.tile_pool(name="io_m", bufs=3))
            io_mu = ctx.enter_context(tc.tile_pool(name="io_mu", bufs=3))
            mids = ctx.enter_context(tc.tile_pool(name="mids", bufs=3))
            mids_big = ctx.enter_context(tc.tile_pool(name="mids_big", bufs=2))
            consts = ctx.enter_context(tc.tile_pool(name="consts", bufs=1))

            # Constant scan mask: 1 everywhere, 0 at each segment's i==0 slot.
            mask = consts.tile([P, FN], f32)
            nc.vector.memset(mask[:], 1.0)
            nc.vector.memset(
                mask[:].rearrange("p (gi n) -> p gi n", gi=g)[:, :, 0:1], 0.0
            )

            for t_rep in range(nt * reps):
                t = t_rep % nt
                q_t = io_q.tile([P, FND], f32)
                nc.sync.dma_start(q_t[:], qv[t])
                v_t = io_v.tile([P, FND], f32)
                nc.sync.dma_start(v_t[:], vv[t])
                m_t = io_m.tile([P, FN], f32)
                nc.sync.dma_start(m_t[:], mv[t])

                # M = segmented cumsum(m): state = mask*state + m
                M_t = mids.tile([P, FN], f32, tag="M")
                # The S2S2D2_STT scan encoding has no sync-wait slots; this
                # 1-element copy absorbs the cross-engine waits (reads m_t,
                # writes M_t) and WAW-orders itself before the scan on DVE.
                nc.vector.tensor_copy(M_t[:, 0:1], m_t[:, 0:1])
                nc.vector.tensor_tensor_scan(
                    M_t[:], mask[:], m_t[:], 0.0, OP.mult, OP.add
                )

                # r = 1/M via exp(-ln M) on ScalarE (keeps division off DVE)
                lnM = mids.tile([P, FN], f32, tag="lnM")
                nc.scalar.activation(lnM[:], M_t[:], AF.Ln)
                r_t = mids.tile([P, FN], f32, tag="r")
                nc.scalar.activation(r_t[:], lnM[:], AF.Exp, scale=-1.0)

                # b = m*r ; a = 1 - b  (a==0 at i==0: the scan reset)
                b_t = mids.tile([P, FN], f32, tag="b")
                nc.vector.tensor_mul(b_t[:], m_t[:], r_t[:])
                a_t = mids.tile([P, FN], f32, tag="a")
                nc.scalar.activation(a_t[:], b_t[:], AF.Identity, bias=1.0, scale=-1.0)

                # mu = G * M
                mu_t = io_mu.tile([P, FN], f32)
                nc.scalar.mul(mu_t[:], M_t[:], G_GRAV)
                nc.scalar.dma_start(muv[t], mu_t[:])

                b_bcast = (
                    b_t[:]
                    .rearrange("p (gi n) -> p gi n", gi=g)
                    .unsqueeze(3)
                    .broadcast_to([P, g, N, D])
                )

                for x_t, out_view, nm in ((q_t, qjv, "q"), (v_t, vjv, "v")):
                    x4 = x_t[:].rearrange("p (gi n d) -> p gi n d", gi=g, n=N)
                    # bx = b * x
                    bx = mids_big.tile([P, FND], f32, tag=f"bx_{nm}")
                    bx4 = bx[:].rearrange("p (gi n d) -> p gi n d", gi=g, n=N)
                    nc.vector.tensor_mul(bx4, x4, b_bcast)
                    # COM via one segmented scan per component d
                    com = mids_big.tile([P, FND], f32, tag=f"com_{nm}")
                    # wait absorber for the scans (see M-scan comment)
                    nc.vector.tensor_copy(com[:, 0:1], a_t[:, 0:1])
                    com_xd = com[:].rearrange("p (x d) -> p x d", d=D)
                    bx_xd = bx[:].rearrange("p (x d) -> p x d", d=D)
                    for d in range(D):
                        nc.vector.tensor_tensor_scan(
                            com_xd[:, :, d],
                            a_t[:],
                            bx_xd[:, :, d],
                            0.0,
                            OP.mult,
                            OP.add,
                        )
                    com4 = com[:].rearrange("p (gi n d) -> p gi n d", gi=g, n=N)
                    # xj[i>=1] = x[i] - COM[i-1]   (in place over x_t)
                    nc.vector.tensor_sub(
                        x4[:, :, 1:, :], x4[:, :, 1:, :], com4[:, :, 0 : N - 1, :]
                    )
                    # xj[0] = COM[N-1]
                    nc.scalar.copy(x4[:, :, 0:1, :], com4[:, :, N - 1 : N, :])
                    nc.sync.dma_start(out_view[t], x_t[:])

    nc.compile()
    return nc


_NC_CACHE: dict = {}


def _get_nc(bc: int, g: int, reps: int = 1) -> bass.Bass:
    key = (bc, g, reps)
    if key not in _NC_CACHE:
        _NC_CACHE[key] = build_nc(bc, g, reps)
    return _NC_CACHE[key]


def run_sharded(m, q, v, g=G_DEFAULT, reps=1, trace=False, **kwargs):
    """Run the SPMD kernel on the 8 cores; returns (qj, vj, mu, results_obj)."""
    m = np.ascontiguousarray(np.asarray(m), dtype=np.float32)
    q = np.ascontiguousarray(np.asarray(q), dtype=np.float32)
    v = np.ascontiguousarray(np.asarray(v), dtype=np.float32)
    bfull = m.shape[0]
    bc = bfull // N_CORES
    in_maps = [
        {
            "m": m[i * bc : (i + 1) * bc],
            "q": q[i * bc : (i + 1) * bc],
            "v": v[i * bc : (i + 1) * bc],
        }
        for i in range(N_CORES)
    ]
    res = run_bass_kernel_spmd(
        _get_nc(bc, g, reps), in_maps, list(range(N_CORES)), trace=trace, **kwargs
    )
    qj = np.concatenate([r["qj"] for r in res.results], axis=0)
    vj = np.concatenate([r["vj"] for r in res.results], axis=0)
    mu = np.concatenate([r["mu"] for r in res.results], axis=0)
    return qj, vj, mu, res


def kernel(m, q, v):
    qj, vj, mu, _ = run_sharded(m, q, v)
    return qj, vj, mu


# revision 18
# speedup vs baseline: 158.5834x; 158.5834x over previous
"""Trainium2 Bass kernel: batched Cartesian -> Jacobi coordinate transform.

Math (per batch element, N = 11 bodies, D = 3):
    M[i]   = cumsum(m)[i]
    COM[i] = (sum_{j<=i} m[j] x[j]) / M[i]
           = a[i] * COM[i-1] + b[i] * x[i],  b[i] = m[i]/M[i], a[i] = 1 - b[i]
    xj[0]  = COM[N-1]          (x is q or v)
    xj[i]  = x[i] - COM[i-1]   (i >= 1)
    mu     = G * M

a[0] == 0 exactly (b[0] = m[0]/M[0] = 1), so the recurrence self-resets at
each batch element's first body.  That lets the whole tile's COM computation
run as a single DVE tensor_tensor_scan per coordinate component d (op0=mult,
op1=add, strided 2-D access pattern), with the reset riding along in data0.

Layout: batch on the 128 SBUF partitions, G batch elements per partition per
tile; each batch element's [N, D] block stays contiguous along the free dim,
so all HBM DMAs are fully contiguous.

Sharding: pure data-parallel over the batch dim across 8 NeuronCores.
"""

import numpy as np

import concourse.bacc as bacc
import concourse.bass as bass
import concourse.mybir as mybir
import concourse.tile as tile
from concourse.bass_utils import run_bass_kernel_spmd

G_GRAV = 2.959122082855911e-4
B_FULL, N, D = 524288, 11, 3
N_CORES = 8
P = 128  # SBUF partitions
BC = B_FULL // N_CORES  # batch per core
G_DEFAULT = 64  # batch elements per partition per tile


class _Bacc(bacc.Bacc):
    """Bacc whose activation-table selection is pinned to
    `natural_log_exp_and_others` — the one set that really contains Ln, Exp,
    Identity and Copy — so the per-tile Ln/Exp + Copy mix doesn't thrash ACT
    table loads (the stock greedy pass alternates natural_log <->
    exp_and_others, ~2.7us per reload)."""

    def insert_act_table_loads(self):
        import bass_rust as _bass_rust
        from concourse.hw_specs import get_activation_tables

        has_activation = any(
            isinstance(i, mybir.InstActivation)
            for b in self.main_func.blocks
            for i in b.instructions
        )
        if not has_activation:
            return
        AF = mybir.ActivationFunctionType
        keep = "natural_log_exp_and_others"
        strip = {AF.Ln, AF.Exp, AF.Identity, AF.Copy}
        tables = [
            (name, funcs if name == keep else (funcs - strip))
            for name, funcs in get_activation_tables(self.m.arch).items()
        ]
        _bass_rust.insert_act_table_loads(self, tables)


def build_nc(bc: int, g: int, reps: int = 1, loop: int = 1) -> bass.Bass:
    """Build the per-core Bass program for a batch shard of `bc` elements.

    reps > 1 unrolls the whole computation (same I/O); loop > 1 wraps it in a
    dynamic For_i. Both are for differential wall-clock timing; results are
    unchanged."""
    nt = bc // (P * g)
    assert nt * P * g == bc, (bc, g)
    f32 = mybir.dt.float32
    AF = mybir.ActivationFunctionType
    OP = mybir.AluOpType
    FN = g * N
    FND = g * N * D

    nc = _Bacc("TRN2")
    m_in = nc.dram_tensor("m", [bc, N], f32, kind="ExternalInput")
    q_in = nc.dram_tensor("q", [bc, N, D], f32, kind="ExternalInput")
    v_in = nc.dram_tensor("v", [bc, N, D], f32, kind="ExternalInput")
    qj_out = nc.dram_tensor("qj", [bc, N, D], f32, kind="ExternalOutput")
    vj_out = nc.dram_tensor("vj", [bc, N, D], f32, kind="ExternalOutput")
    mu_out = nc.dram_tensor("mu", [bc, N], f32, kind="ExternalOutput")

    # [t, p, free] views; batch index b = (t*128 + p)*g + gi  (fully contiguous)
    mv = m_in[:].rearrange("(t p gi) n -> t p (gi n)", t=nt, p=P, gi=g)
    qv = q_in[:].rearrange("(t p gi) n d -> t p (gi n d)", t=nt, p=P, gi=g)
    vv = v_in[:].rearrange("(t p gi) n d -> t p (gi n d)", t=nt, p=P, gi=g)
    qjv = qj_out[:].rearrange("(t p gi) n d -> t p (gi n d)", t=nt, p=P, gi=g)
    vjv = vj_out[:].rearrange("(t p gi) n d -> t p (gi n d)", t=nt, p=P, gi=g)
    muv = mu_out[:].rearrange("(t p gi) n -> t p (gi n)", t=nt, p=P, gi=g)

    with tile.TileContext(nc) as tc:
        from contextlib import ExitStack

        with ExitStack() as ctx:
            io_q = ctx.enter_context(tc.tile_pool(name="io_q", bufs=3))
            io_v = ctx.enter_context(tc.tile_pool(name="io_v", bufs=3))
            io_m = ctx.enter_context(tc.tile_pool(name="io_m", bufs=3))
            io_mu = ctx.enter_context(tc.tile_pool(name="io_mu", bufs=3))
            mids = ctx.enter_context(tc.tile_pool(name="mids", bufs=3))
            mids_big = ctx.enter_context(tc.tile_pool(name="mids_big", bufs=2))
            consts = ctx.enter_context(tc.tile_pool(name="consts", bufs=1))

            # Constant scan mask: 1 everywhere, 0 at each segment's i==0 slot.
            mask = consts.tile([P, FN], f32)
            nc.vector.memset(mask[:], 1.0)
            nc.vector.memset(
                mask[:].rearrange("p (gi n) -> p gi n", gi=g)[:, :, 0:1], 0.0
            )

            from contextlib import nullcontext

            loop_cm = tc.For_i(0, loop, 1) if loop > 1 else nullcontext()
            with loop_cm:
                body(nc, tc, reps, nt, g, io_q, io_v, io_m, io_mu, mids,
                     mids_big, mask, qv, vv, mv, qjv, vjv, muv)

    nc.compile()
    return nc


def body(nc, tc, reps, nt, g, io_q, io_v, io_m, io_mu, mids, mids_big, mask,
         qv, vv, mv, qjv, vjv, muv):
    f32 = mybir.dt.float32
    AF = mybir.ActivationFunctionType
    OP = mybir.AluOpType
    FN = g * N
    FND = g * N * D
    if True:
        if True:
            for t_rep in range(nt * reps):
                t = t_rep % nt
                q_t = io_q.tile([P, FND], f32)
                nc.sync.dma_start(q_t[:], qv[t])
                v_t = io_v.tile([P, FND], f32)
                nc.sync.dma_start(v_t[:], vv[t])
                m_t = io_m.tile([P, FN], f32)
                nc.sync.dma_start(m_t[:], mv[t])

                # M = segmented cumsum(m): state = mask*state + m
                M_t = mids.tile([P, FN], f32, tag="M")
                # The S2S2D2_STT scan encoding has no sync-wait slots; this
                # 1-element copy absorbs the cross-engine waits (reads m_t,
                # writes M_t) and WAW-orders itself before the scan on DVE.
                nc.vector.tensor_copy(M_t[:, 0:1], m_t[:, 0:1])
                nc.vector.tensor_tensor_scan(
                    M_t[:], mask[:], m_t[:], 0.0, OP.mult, OP.add
                )

                # r = 1/M via exp(-ln M) on ScalarE (keeps division off DVE)
                lnM = mids.tile([P, FN], f32, tag="lnM")
                nc.scalar.activation(lnM[:], M_t[:], AF.Ln)
                r_t = mids.tile([P, FN], f32, tag="r")
                nc.scalar.activation(r_t[:], lnM[:], AF.Exp, scale=-1.0)

                # b = m*r ; a = 1 - b  (a==0 at i==0: the scan reset)
                b_t = mids.tile([P, FN], f32, tag="b")
                nc.vector.tensor_mul(b_t[:], m_t[:], r_t[:])
                a_t = mids.tile([P, FN], f32, tag="a")
                nc.scalar.activation(a_t[:], b_t[:], AF.Identity, bias=1.0, scale=-1.0)

                # mu = G * M
                mu_t = io_mu.tile([P, FN], f32)
                nc.scalar.mul(mu_t[:], M_t[:], G_GRAV)
                nc.scalar.dma_start(muv[t], mu_t[:])

                b_bcast = (
                    b_t[:]
                    .rearrange("p (gi n) -> p gi n", gi=g)
                    .unsqueeze(3)
                    .broadcast_to([P, g, N, D])
                )

                for x_t, out_view, nm in ((q_t, qjv, "q"), (v_t, vjv, "v")):
                    x4 = x_t[:].rearrange("p (gi n d) -> p gi n d", gi=g, n=N)
                    # bx = b * x
                    bx = mids_big.tile([P, FND], f32, tag=f"bx_{nm}")
                    bx4 = bx[:].rearrange("p (gi n d) -> p gi n d", gi=g, n=N)
                    nc.vector.tensor_mul(bx4, x4, b_bcast)
                    # COM via one segmented scan per component d
                    com = mids_big.tile([P, FND], f32, tag=f"com_{nm}")
                    # wait absorber for the scans (see M-scan comment)
                    nc.vector.tensor_copy(com[:, 0:1], a_t[:, 0:1])
                    com_xd = com[:].rearrange("p (x d) -> p x d", d=D)
                    bx_xd = bx[:].rearrange("p (x d) -> p x d", d=D)
                    for d in range(D):
                        nc.vector.tensor_tensor_scan(
                            com_xd[:, :, d],
                            a_t[:],
                            bx_xd[:, :, d],
                            0.0,
                            OP.mult,
                            OP.add,
                        )
                    com4 = com[:].rearrange("p (gi n d) -> p gi n d", gi=g, n=N)
                    # xj[i>=1] = x[i] - COM[i-1]   (in place over x_t)
                    nc.vector.tensor_sub(
                        x4[:, :, 1:, :], x4[:, :, 1:, :], com4[:, :, 0 : N - 1, :]
                    )
                    # xj[0] = COM[N-1]
                    nc.scalar.copy(x4[:, :, 0:1, :], com4[:, :, N - 1 : N, :])
                    nc.sync.dma_start(out_view[t], x_t[:])


_NC_CACHE: dict = {}


def _get_nc(bc: int, g: int, reps: int = 1, loop: int = 1) -> bass.Bass:
    key = (bc, g, reps, loop)
    if key not in _NC_CACHE:
        _NC_CACHE[key] = build_nc(bc, g, reps, loop)
    return _NC_CACHE[key]


def run_sharded(m, q, v, g=G_DEFAULT, reps=1, trace=False, **kwargs):
    """Run the SPMD kernel on the 8 cores; returns (qj, vj, mu, results_obj)."""
    m = np.ascontiguousarray(np.asarray(m), dtype=np.float32)
    q = np.ascontiguousarray(np.asarray(q), dtype=np.float32)
    v = np.ascontiguousarray(np.asarray(v), dtype=np.float32)
    bfull = m.shape[0]
    bc = bfull // N_CORES
    in_maps = [
        {
            "m": m[i * bc : (i + 1) * bc],
            "q": q[i * bc : (i + 1) * bc],
            "v": v[i * bc : (i + 1) * bc],
        }
        for i in range(N_CORES)
    ]
    res = run_bass_kernel_spmd(
        _get_nc(bc, g, reps), in_maps, list(range(N_CORES)), trace=trace, **kwargs
    )
    qj = np.concatenate([r["qj"] for r in res.results], axis=0)
    vj = np.concatenate([r["vj"] for r in res.results], axis=0)
    mu = np.concatenate([r["mu"] for r in res.results], axis=0)
    return qj, vj, mu, res


def kernel(m, q, v):
    qj, vj, mu, _ = run_sharded(m, q, v)
    return qj, vj, mu


# revision 24
# speedup vs baseline: 264.8506x; 1.6701x over previous
"""Trainium2 Bass kernel: batched Cartesian -> Jacobi coordinate transform.

Math (per batch element, N = 11 bodies, D = 3):
    M[i]   = cumsum(m)[i]
    COM[i] = (sum_{j<=i} m[j] x[j]) / M[i]
           = a[i] * COM[i-1] + b[i] * x[i],  b[i] = m[i]/M[i], a[i] = 1 - b[i]
    xj[0]  = COM[N-1]          (x is q or v)
    xj[i]  = x[i] - COM[i-1]   (i >= 1)
    mu     = G * M

a[0] == 0 exactly (b[0] = m[0]/M[0] = 1), so the recurrence self-resets at
each batch element's first body.  That lets the whole tile's COM computation
run as a single DVE tensor_tensor_scan per coordinate component d (op0=mult,
op1=add, strided 2-D access pattern), with the reset riding along in data0.

Layout: batch on the 128 SBUF partitions, G batch elements per partition per
tile; each batch element's [N, D] block stays contiguous along the free dim,
so all HBM DMAs are fully contiguous.

Sharding: pure data-parallel over the batch dim across 8 NeuronCores.
"""

import numpy as np

import concourse.bacc as bacc
import concourse.bass as bass
import concourse.mybir as mybir
import concourse.tile as tile
from concourse.bass_utils import run_bass_kernel_spmd

G_GRAV = 2.959122082855911e-4
B_FULL, N, D = 524288, 11, 3
N_CORES = 8
P = 128  # SBUF partitions
BC = B_FULL // N_CORES  # batch per core
G_DEFAULT = 64  # batch elements per partition per tile


class _Bacc(bacc.Bacc):
    """Bacc whose activation-table selection is pinned to
    `natural_log_exp_and_others` — the one set that really contains Ln, Exp,
    Identity and Copy — so the per-tile Ln/Exp + Copy mix doesn't thrash ACT
    table loads (the stock greedy pass alternates natural_log <->
    exp_and_others, ~2.7us per reload)."""

    def insert_act_table_loads(self):
        import bass_rust as _bass_rust
        from concourse.hw_specs import get_activation_tables

        has_activation = any(
            isinstance(i, mybir.InstActivation)
            for b in self.main_func.blocks
            for i in b.instructions
        )
        if not has_activation:
            return
        AF = mybir.ActivationFunctionType
        keep = "natural_log_exp_and_others"
        strip = {AF.Ln, AF.Exp, AF.Identity, AF.Copy}
        tables = [
            (name, funcs if name == keep else (funcs - strip))
            for name, funcs in get_activation_tables(self.m.arch).items()
        ]
        _bass_rust.insert_act_table_loads(self, tables)


def build_nc(
    bc: int, g: int, reps: int = 1, loop: int = 1, mode: str = "full"
) -> bass.Bass:
    """Build the per-core Bass program for a batch shard of `bc` elements.

    reps > 1 unrolls the whole computation (same I/O); loop > 1 wraps it in a
    dynamic For_i. Both are for differential wall-clock timing; results are
    unchanged.  mode: 'full' | 'dma' (loads+stores only) | 'compute' (no DMA,
    engines run on garbage) — for bottleneck decomposition."""
    nt = bc // (P * g)
    assert nt * P * g == bc, (bc, g)
    f32 = mybir.dt.float32
    AF = mybir.ActivationFunctionType
    OP = mybir.AluOpType
    FN = g * N
    FND = g * N * D

    nc = _Bacc("TRN2")
    m_in = nc.dram_tensor("m", [bc, N], f32, kind="ExternalInput")
    q_in = nc.dram_tensor("q", [bc, N, D], f32, kind="ExternalInput")
    v_in = nc.dram_tensor("v", [bc, N, D], f32, kind="ExternalInput")
    qj_out = nc.dram_tensor("qj", [bc, N, D], f32, kind="ExternalOutput")
    vj_out = nc.dram_tensor("vj", [bc, N, D], f32, kind="ExternalOutput")
    mu_out = nc.dram_tensor("mu", [bc, N], f32, kind="ExternalOutput")

    # [t, p, free] views; batch index b = (t*128 + p)*g + gi  (fully contiguous)
    mv = m_in[:].rearrange("(t p gi) n -> t p (gi n)", t=nt, p=P, gi=g)
    qv = q_in[:].rearrange("(t p gi) n d -> t p (gi n d)", t=nt, p=P, gi=g)
    vv = v_in[:].rearrange("(t p gi) n d -> t p (gi n d)", t=nt, p=P, gi=g)
    qjv = qj_out[:].rearrange("(t p gi) n d -> t p (gi n d)", t=nt, p=P, gi=g)
    vjv = vj_out[:].rearrange("(t p gi) n d -> t p (gi n d)", t=nt, p=P, gi=g)
    muv = mu_out[:].rearrange("(t p gi) n -> t p (gi n)", t=nt, p=P, gi=g)

    with tile.TileContext(nc) as tc:
        from contextlib import ExitStack

        with ExitStack() as ctx:
            io_q = ctx.enter_context(tc.tile_pool(name="io_q", bufs=3))
            io_v = ctx.enter_context(tc.tile_pool(name="io_v", bufs=3))
            io_m = ctx.enter_context(tc.tile_pool(name="io_m", bufs=3))
            io_mu = ctx.enter_context(tc.tile_pool(name="io_mu", bufs=3))
            mids = ctx.enter_context(tc.tile_pool(name="mids", bufs=3))
            mids_big = ctx.enter_context(tc.tile_pool(name="mids_big", bufs=2))
            consts = ctx.enter_context(tc.tile_pool(name="consts", bufs=1))

            # Constant scan mask: 1 everywhere, 0 at each segment's i==0 slot.
            mask = consts.tile([P, FN], f32)
            nc.vector.memset(mask[:], 1.0)
            nc.vector.memset(
                mask[:].rearrange("p (gi n) -> p gi n", gi=g)[:, :, 0:1], 0.0
            )

            from contextlib import nullcontext

            loop_cm = tc.For_i(0, loop, 1) if loop > 1 else nullcontext()
            with loop_cm:
                body(nc, tc, reps, nt, g, io_q, io_v, io_m, io_mu, mids,
                     mids_big, mask, qv, vv, mv, qjv, vjv, muv, mode)

    nc.compile()
    return nc


def body(nc, tc, reps, nt, g, io_q, io_v, io_m, io_mu, mids, mids_big, mask,
         qv, vv, mv, qjv, vjv, muv, mode="full"):
    f32 = mybir.dt.float32
    AF = mybir.ActivationFunctionType
    OP = mybir.AluOpType
    FN = g * N
    FND = g * N * D
    dma = mode in ("full", "dma")
    compute = mode in ("full", "compute")
    if True:
        if True:
            for t_rep in range(nt * reps):
                t = t_rep % nt
                q_t = io_q.tile([P, FND], f32)
                v_t = io_v.tile([P, FND], f32)
                m_t = io_m.tile([P, FN], f32)
                if dma:
                    nc.sync.dma_start(q_t[:], qv[t])
                    nc.sync.dma_start(v_t[:], vv[t])
                    nc.sync.dma_start(m_t[:], mv[t])
                if not compute:
                    mu_t = io_mu.tile([P, FN], f32)
                    nc.sync.dma_start(qjv[t], q_t[:])
                    nc.sync.dma_start(vjv[t], v_t[:])
                    nc.scalar.dma_start(muv[t], m_t[:])
                    continue

                # M = segmented cumsum(m): state = mask*state + m
                M_t = mids.tile([P, FN], f32, tag="M")
                # The S2S2D2_STT scan encoding has no sync-wait slots; this
                # 1-element copy absorbs the cross-engine waits (reads m_t,
                # writes M_t) and WAW-orders itself before the scan on DVE.
                nc.vector.tensor_copy(M_t[:, 0:1], m_t[:, 0:1])
                nc.vector.tensor_tensor_scan(
                    M_t[:], mask[:], m_t[:], 0.0, OP.mult, OP.add
                )

                # r = 1/M via exp(-ln M) on ScalarE (keeps division off DVE)
                lnM = mids.tile([P, FN], f32, tag="lnM")
                nc.scalar.activation(lnM[:], M_t[:], AF.Ln)
                r_t = mids.tile([P, FN], f32, tag="r")
                nc.scalar.activation(r_t[:], lnM[:], AF.Exp, scale=-1.0)

                # b = m*r ; a = 1 - b  (a==0 at i==0: the scan reset)
                b_t = mids.tile([P, FN], f32, tag="b")
                nc.vector.tensor_mul(b_t[:], m_t[:], r_t[:])
                a_t = mids.tile([P, FN], f32, tag="a")
                nc.scalar.activation(a_t[:], b_t[:], AF.Identity, bias=1.0, scale=-1.0)

                # mu = G * M
                mu_t = io_mu.tile([P, FN], f32)
                nc.scalar.mul(mu_t[:], M_t[:], G_GRAV)
                if dma:
                    nc.scalar.dma_start(muv[t], mu_t[:])

                b_bcast = (
                    b_t[:]
                    .rearrange("p (gi n) -> p gi n", gi=g)
                    .unsqueeze(3)
                    .broadcast_to([P, g, N, D])
                )

                for x_t, out_view, nm in ((q_t, qjv, "q"), (v_t, vjv, "v")):
                    x4 = x_t[:].rearrange("p (gi n d) -> p gi n d", gi=g, n=N)
                    # bx = b * x
                    bx = mids_big.tile([P, FND], f32, tag=f"bx_{nm}")
                    bx4 = bx[:].rearrange("p (gi n d) -> p gi n d", gi=g, n=N)
                    nc.vector.tensor_mul(bx4, x4, b_bcast)
                    # COM via one segmented scan per component d
                    com = mids_big.tile([P, FND], f32, tag=f"com_{nm}")
                    # wait absorber for the scans (see M-scan comment)
                    nc.vector.tensor_copy(com[:, 0:1], a_t[:, 0:1])
                    com_xd = com[:].rearrange("p (x d) -> p x d", d=D)
                    bx_xd = bx[:].rearrange("p (x d) -> p x d", d=D)
                    for d in range(D):
                        nc.vector.tensor_tensor_scan(
                            com_xd[:, :, d],
                            a_t[:],
                            bx_xd[:, :, d],
                            0.0,
                            OP.mult,
                            OP.add,
                        )
                    com4 = com[:].rearrange("p (gi n d) -> p gi n d", gi=g, n=N)
                    # xj[i>=1] = x[i] - COM[i-1]   (in place over x_t)
                    nc.vector.tensor_sub(
                        x4[:, :, 1:, :], x4[:, :, 1:, :], com4[:, :, 0 : N - 1, :]
                    )
                    # xj[0] = COM[N-1]
                    nc.scalar.copy(x4[:, :, 0:1, :], com4[:, :, N - 1 : N, :])
                    if dma:
                        nc.sync.dma_start(out_view[t], x_t[:])


_NC_CACHE: dict = {}


def _get_nc(
    bc: int, g: int, reps: int = 1, loop: int = 1, mode: str = "full"
) -> bass.Bass:
    key = (bc, g, reps, loop, mode)
    if key not in _NC_CACHE:
        _NC_CACHE[key] = build_nc(bc, g, reps, loop, mode)
    return _NC_CACHE[key]


def run_sharded(m, q, v, g=G_DEFAULT, reps=1, trace=False, **kwargs):
    """Run the SPMD kernel on the 8 cores; returns (qj, vj, mu, results_obj)."""
    m = np.ascontiguousarray(np.asarray(m), dtype=np.float32)
    q = np.ascontiguousarray(np.asarray(q), dtype=np.float32)
    v = np.ascontiguousarray(np.asarray(v), dtype=np.float32)
    bfull = m.shape[0]
    bc = bfull // N_CORES
    in_maps = [
        {
            "m": m[i * bc : (i + 1) * bc],
            "q": q[i * bc : (i + 1) * bc],
            "v": v[i * bc : (i + 1) * bc],
        }
        for i in range(N_CORES)
    ]
    res = run_bass_kernel_spmd(
        _get_nc(bc, g, reps), in_maps, list(range(N_CORES)), trace=trace, **kwargs
    )
    qj = np.concatenate([r["qj"] for r in res.results], axis=0)
    vj = np.concatenate([r["vj"] for r in res.results], axis=0)
    mu = np.concatenate([r["mu"] for r in res.results], axis=0)
    return qj, vj, mu, res


def kernel(m, q, v):
    qj, vj, mu, _ = run_sharded(m, q, v)
    return qj, vj, mu
